# revision 1
# baseline (speedup 1.0000x reference)
"""Trainium2 Bass kernel for nn_Nonlocal (sparse_attention, non-local style attention).

Math (per batch b):
  xn  = instance_norm(content);  sn = instance_norm(style)
  Th  = theta_w @ xn + theta_b          (256, 4096)
  Ph  = phi_w   @ sn + phi_b            (256, 4096)
  g   = g_w @ fusion_style + g_b        (256, 4096)
  f[l,m] = sum_k scale[k]^2 * <Th[:, N_k(l)], Ph[:, N_k(m)]>   (4096, 4096)
           where N_k = 3x3 reflect-padded neighborhood shift
  P = softmax_rows(f);  y = P @ g^T;  out = W_w @ y^T + W_b    (512, 4096)

Sharding: 8 cores = 2 batches x 4 query-row shards (1024 rows of f each).
Instance-norm is folded into the conv weights on the host. The 3x3 shifts are
folded into matmul access patterns: j-axis (within-64 with reflection) via
materialized shifted copies of Th/Ph, i-axis (+-64) via column offsets over
host-reflect-extended windows. Softmax is computed flash-style over four
1024-column PSUM quarters. PV uses PE transposes of P and bf16 matmuls.
"""
import numpy as np

import concourse.bass as bass
import concourse.mybir as mybir
from concourse import bacc
from concourse.bass_utils import run_bass_kernel_spmd
from concourse.tile import TileContext
from concourse.masks import make_identity

F32 = mybir.dt.float32
F32R = mybir.dt.float32r
BF16 = mybir.dt.bfloat16

B, C, H, Wd = 2, 512, 64, 64
HW = H * Wd          # 4096
IC = 256
L = HW // 4          # 1024 query rows per core
WIN = L + 2 * 64     # 1152 theta window cols
EXT = HW + 2 * 64    # 4224 phi extended cols
NT = L // 128        # 8 tiles per core
NQ = 4               # psum quarters per tile (1024 cols each)
QC = HW // NQ        # 1024


def _build_program(nt_max=NT, do_tail=True):
    nc = bacc.Bacc("TRN2", target_bir_lowering=False, debug=False, num_devices=8)

    cw_d = nc.dram_tensor("cw", [4, 128, WIN], F32R, kind="ExternalInput")
    st_d = nc.dram_tensor("st", [4, 128, EXT], F32R, kind="ExternalInput")
    fu_d = nc.dram_tensor("fu", [4, 128, HW], F32R, kind="ExternalInput")
    thw_d = nc.dram_tensor("thw", [4, 128, IC], F32R, kind="ExternalInput")
    phw_d = nc.dram_tensor("phw", [4, 128, IC], F32R, kind="ExternalInput")
    gw_d = nc.dram_tensor("gw", [4, 128, IC], F32R, kind="ExternalInput")
    ww_d = nc.dram_tensor("ww", [2, 128, C], F32R, kind="ExternalInput")
    bth_d = nc.dram_tensor("bth", [2, 128, 1], F32, kind="ExternalInput")
    bph_d = nc.dram_tensor("bph", [2, 128, 1], F32, kind="ExternalInput")
    bout_d = nc.dram_tensor("bout", [4, 128, 1], F32, kind="ExternalInput")
    o_d = nc.dram_tensor("o", [4, 128, L], F32, kind="ExternalOutput")

    with TileContext(nc) as tc:
        with tc.tile_pool(name="const", bufs=1) as constp, \
             tc.tile_pool(name="persist", bufs=1) as persist, \
             tc.tile_pool(name="work", bufs=2) as work, \
             tc.tile_pool(name="stats", bufs=3) as stats, \
             tc.tile_pool(name="fqp", bufs=2, space="PSUM") as fqp, \
             tc.tile_pool(name="ptp", bufs=2, space="PSUM") as ptp, \
             tc.tile_pool(name="yp", bufs=2, space="PSUM") as yp:

            ident = constp.tile([128, 128], F32)
            make_identity(nc, ident)
            thw = constp.tile([128, 4, IC], F32R)
            phw = constp.tile([128, 4, IC], F32R)
            gw = constp.tile([128, 4, IC], F32R)
            ww = constp.tile([128, 2, C], F32R)
            bth = constp.tile([128, 2, 1], F32)
            bph = constp.tile([128, 2, 1], F32)
            bout = constp.tile([128, 4, 1], F32)
            for kk in range(4):
                nc.sync.dma_start(out=thw[:, kk, :], in_=thw_d[kk])
                nc.sync.dma_start(out=phw[:, kk, :], in_=phw_d[kk])
                nc.sync.dma_start(out=gw[:, kk, :], in_=gw_d[kk])
                nc.sync.dma_start(out=bout[:, kk, :], in_=bout_d[kk])
            for kk in range(2):
                nc.sync.dma_start(out=ww[:, kk, :], in_=ww_d[kk])
                nc.sync.dma_start(out=bth[:, kk, :], in_=bth_d[kk])
                nc.sync.dma_start(out=bph[:, kk, :], in_=bph_d[kk])

            # persistent big tensors
            th_j = persist.tile([128, 2, 3, WIN], F32R)     # theta, j-shifted x3
            ph_j = persist.tile([128, 2, 3, EXT], F32R)     # phi, j-shifted x3
            gt = persist.tile([128, 32, IC], BF16)          # g^T chunks (m-part)
            yT = persist.tile([128, 2, L], F32R)            # y^T accumulator

            # ---- stage A: gT from fusion_style (streamed in 1024-col groups) ----
            with tc.tile_pool(name="ful", bufs=1) as ful:
                for mg in range(4):
                    fu = ful.tile([128, 4, 1024], F32R, tag="fu")
                    for kk in range(4):
                        nc.sync.dma_start(out=fu[:, kk, :],
                                          in_=fu_d[kk][:, 1024 * mg:1024 * (mg + 1)])
                    for jj in range(8):
                        j = 8 * mg + jj
                        ps = yp.tile([128, IC], F32, tag="yps")
                        for kk in range(4):
                            nc.tensor.matmul(ps, fu[:, kk, 128 * jj:128 * (jj + 1)],
                                             gw[:, kk, :], start=(kk == 0), stop=(kk == 3))
                        if j % 2 == 0:
                            nc.vector.tensor_copy(gt[:, j, :], ps)
                        else:
                            nc.scalar.copy(gt[:, j, :], ps)

            # ---- stage B: phi (with bias) and its j-shifted copies ----
            with tc.tile_pool(name="stl", bufs=1) as stl:
                for mg in range(5):
                    g0 = 1024 * mg
                    gw_cols = min(1024, EXT - g0)
                    st = stl.tile([128, 4, 1024], F32R, tag="st")
                    for kk in range(4):
                        nc.sync.dma_start(out=st[:, kk, 0:gw_cols],
                                          in_=st_d[kk][:, g0:g0 + gw_cols])
                    for oc in range(2):
                        n0 = 0
                        while n0 < gw_cols:
                            nn = min(512, gw_cols - n0)
                            ps = fqp.tile([128, 1024], F32, tag="fq")
                            for kk in range(4):
                                nc.tensor.matmul(ps[:, 0:nn],
                                                 phw[:, kk, 128 * oc:128 * (oc + 1)],
                                                 st[:, kk, n0:n0 + nn],
                                                 start=(kk == 0), stop=(kk == 3))
                            nc.vector.tensor_scalar_add(
                                ph_j[:, oc, 1, g0 + n0:g0 + n0 + nn],
                                ps[:, 0:nn], bph[:, oc, :])
                            n0 += nn
                # j-shifted copies (within 64-col blocks, reflect at edges)
                for oc in range(2):
                    src = ph_j[:, oc, 1, :].rearrange("p (b j) -> p b j", j=64)
                    for dj, dst_i in ((0, 0), (2, 2)):
                        dst = ph_j[:, oc, dst_i, :].rearrange("p (b j) -> p b j", j=64)
                        if dj == 0:
                            nc.vector.tensor_copy(dst[:, :, 1:64], src[:, :, 0:63])
                            nc.scalar.copy(dst[:, :, 0:1], src[:, :, 1:2])
                        else:
                            nc.vector.tensor_copy(dst[:, :, 0:63], src[:, :, 1:64])
                            nc.scalar.copy(dst[:, :, 63:64], src[:, :, 62:63])

            # ---- stage C: theta (with bias) and its j-shifted copies ----
            with tc.tile_pool(name="cwl", bufs=1) as cwl:
                for mg in range(2):
                    g0 = 1024 * mg
                    gw_cols = min(1024, WIN - g0)
                    cwt = cwl.tile([128, 4, 1024], F32R, tag="cwt")
                    for kk in range(4):
                        nc.sync.dma_start(out=cwt[:, kk, 0:gw_cols],
                                          in_=cw_d[kk][:, g0:g0 + gw_cols])
                    for oc in range(2):
                        n0 = 0
                        while n0 < gw_cols:
                            nn = min(512, gw_cols - n0)
                            ps = fqp.tile([128, 1024], F32, tag="fq")
                            for kk in range(4):
                                nc.tensor.matmul(ps[:, 0:nn],
                                                 thw[:, kk, 128 * oc:128 * (oc + 1)],
                                                 cwt[:, kk, n0:n0 + nn],
                                                 start=(kk == 0), stop=(kk == 3))
                            nc.vector.tensor_scalar_add(
                                th_j[:, oc, 1, g0 + n0:g0 + n0 + nn],
                                ps[:, 0:nn], bth[:, oc, :])
                            n0 += nn
                for oc in range(2):
                    src = th_j[:, oc, 1, :].rearrange("p (b j) -> p b j", j=64)
                    for dj, dst_i in ((0, 0), (2, 2)):
                        dst = th_j[:, oc, dst_i, :].rearrange("p (b j) -> p b j", j=64)
                        if dj == 0:
                            nc.vector.tensor_copy(dst[:, :, 1:64], src[:, :, 0:63])
                            nc.scalar.copy(dst[:, :, 0:1], src[:, :, 1:2])
                        else:
                            nc.vector.tensor_copy(dst[:, :, 0:63], src[:, :, 1:64])
                            nc.scalar.copy(dst[:, :, 63:64], src[:, :, 62:63])

            # ---- main loop over 8 query tiles ----
            for t in range(nt_max):
                negM = stats.tile([128, 1], F32, tag="negM")
                s_run = stats.tile([128, 1], F32, tag="s_run")
                y_sb = work.tile([128, IC], F32, tag="y_sb")
                for q in range(NQ):
                    fq = fqp.tile([128, QC], F32, tag="fq")
                    for nn in range(2):
                        cs = slice(512 * nn, 512 * (nn + 1))
                        first = True
                        for dj in range(3):
                            for di in range(3):
                                for cc in range(2):
                                    last = (dj == 2 and di == 2 and cc == 1)
                                    nc.tensor.matmul(
                                        fq[:, cs],
                                        th_j[:, cc, dj, 128 * t + 64 * di:
                                             128 * t + 64 * di + 128],
                                        ph_j[:, cc, dj, 64 * di + QC * q + 512 * nn:
                                             64 * di + QC * q + 512 * (nn + 1)],
                                        start=first, stop=last)
                                    first = False
                    # flash-style softmax over quarters
                    negmq = stats.tile([128, 1], F32, tag="negmq")
                    nc.vector.tensor_reduce(negmq, fq, axis=mybir.AxisListType.X,
                                            op=mybir.AluOpType.max, negate=True)
                    sq = stats.tile([128, 1], F32, tag="sq")
                    pq = work.tile([128, QC], F32, tag="pq")
                    if q == 0:
                        nc.vector.tensor_copy(negM, negmq)
                        nc.scalar.activation(pq, fq, mybir.ActivationFunctionType.Exp,
                                             bias=negM, scale=1.0, accum_out=s_run)
                    else:
                        posM_old = stats.tile([128, 1], F32, tag="posM")
                        nc.vector.tensor_scalar_mul(posM_old, negM, -1.0)
                        nc.vector.tensor_tensor(negM, negM, negmq,
                                                op=mybir.AluOpType.min)
                        cfac = stats.tile([128, 1], F32, tag="cfac")
                        nc.scalar.activation(cfac, negM,
                                             mybir.ActivationFunctionType.Exp,
                                             bias=posM_old, scale=1.0)
                        nc.scalar.activation(pq, fq, mybir.ActivationFunctionType.Exp,
                                             bias=negM, scale=1.0, accum_out=sq)
                        nc.vector.tensor_scalar_mul(s_run, s_run, cfac)
                        nc.vector.tensor_tensor(s_run, s_run, sq,
                                                op=mybir.AluOpType.add)
                        nc.vector.tensor_scalar_mul(y_sb, y_sb, cfac)
                    # transpose P quarter + PV partial
                    y_ps = yp.tile([128, IC], F32, tag="yps")
                    ptsb = work.tile([128, 8, 128], BF16, tag="ptsb")
                    for j in range(8):
                        pt_ps = ptp.tile([128, 128], F32, tag="pt")
                        nc.tensor.transpose(pt_ps, pq[:, 128 * j:128 * (j + 1)], ident)
                        if j % 2 == 0:
                            nc.vector.tensor_copy(ptsb[:, j, :], pt_ps)
                        else:
                            nc.scalar.copy(ptsb[:, j, :], pt_ps)
                    for j in range(8):
                        nc.tensor.matmul(y_ps, ptsb[:, j, :], gt[:, 8 * q + j, :],
                                         start=(j == 0), stop=(j == 7))
                    if q == 0:
                        nc.vector.tensor_copy(y_sb, y_ps)
                    else:
                        nc.vector.tensor_tensor(y_sb, y_sb, y_ps,
                                                op=mybir.AluOpType.add)
                # normalize and transpose y into yT
                rec = stats.tile([128, 1], F32, tag="rec")
                nc.vector.reciprocal(rec, s_run)
                yn = work.tile([128, IC], F32, tag="yn")
                nc.vector.tensor_scalar_mul(yn, y_sb, rec)
                for oc in range(2):
                    yt_ps = ptp.tile([128, 128], F32, tag="pt")
                    nc.tensor.transpose(yt_ps, yn[:, 128 * oc:128 * (oc + 1)], ident)
                    nc.vector.tensor_copy(yT[:, oc, 128 * t:128 * (t + 1)], yt_ps)

            # ---- tail: W conv + bias + store ----
            if not do_tail:
                with tc.tile_pool(name="outp", bufs=2) as outp:
                    ot = outp.tile([128, L], F32, tag="ot")
                    nc.vector.tensor_copy(ot, th_j[:, 0, 1, 0:L])
                    for mo in range(4):
                        nc.sync.dma_start(out=o_d[mo], in_=ot)
            if do_tail:
              with tc.tile_pool(name="outp", bufs=2) as outp:
                  for mo in range(4):
                      ot = outp.tile([128, L], F32, tag="ot")
                      for nl in range(2):
                          ps = fqp.tile([128, QC], F32, tag="fq")
                          for kk in range(2):
                              nc.tensor.matmul(ps[:, 0:512],
                                               ww[:, kk, 128 * mo:128 * (mo + 1)],
                                               yT[:, kk, 512 * nl:512 * (nl + 1)],
                                               start=(kk == 0), stop=(kk == 1))
                          nc.scalar.activation(ot[:, 512 * nl:512 * (nl + 1)],
                                               ps[:, 0:512],
                                               mybir.ActivationFunctionType.Identity,
                                               bias=bout[:, mo, :], scale=1.0)
                      nc.sync.dma_start(out=o_d[mo], in_=ot)

    nc.compile()
    return nc


_PROG = None


def _reflect_idx(i, n):
    if i < 0:
        return -i
    if i >= n:
        return 2 * (n - 1) - i
    return i


def _host_prep(inputs):
    EPS = 1e-5
    content = np.asarray(inputs["content"], np.float32)
    style = np.asarray(inputs["style"], np.float32)
    fusion = np.asarray(inputs["fusion_style"], np.float32)
    theta_w = np.asarray(inputs["theta_w"], np.float32)
    theta_b = np.asarray(inputs["theta_b"], np.float32)
    phi_w = np.asarray(inputs["phi_w"], np.float32)
    phi_b = np.asarray(inputs["phi_b"], np.float32)
    g_w = np.asarray(inputs["g_w"], np.float32)
    g_b = np.asarray(inputs["g_b"], np.float32)
    W_w = np.asarray(inputs["W_w"], np.float32)
    W_b = np.asarray(inputs["W_b"], np.float32)
    scale = np.asarray(inputs["scale"], np.float32)

    s2 = scale.astype(np.float64) ** 2
    if not np.allclose(s2, s2[0]):
        raise NotImplementedError("non-uniform ContextAtten scale not supported")
    s0 = float(s2[0])

    cf = content.reshape(B, C, HW)
    sf = style.reshape(B, C, HW)
    ff = fusion.reshape(B, C, HW)
    mu_c = cf.mean(-1)
    var_c = cf.var(-1, ddof=1)
    rstd_c = 1.0 / np.sqrt(var_c + EPS)
    mu_s = sf.mean(-1)
    var_s = sf.var(-1, ddof=1)
    rstd_s = 1.0 / np.sqrt(var_s + EPS)

    # reflect-extended column index maps
    def colmap(lo, hi):
        idx = np.empty(hi - lo, np.int64)
        for p, lw in enumerate(range(lo, hi)):
            i1, j1 = divmod(lw, 64)
            idx[p] = _reflect_idx(i1, 64) * 64 + j1
        return idx

    ext_map = colmap(-64, HW + 64)             # 4224

    in_maps = []
    for r in range(8):
        b, sh = divmod(r, 4)
        q0 = sh * L
        win_map = colmap(q0 - 64, q0 + L + 64)  # 1152
        # fold instance norm (and uniform scale**2 on theta) into weights
        thw = (theta_w * rstd_c[b][None, :] * s0).T.copy()       # (512, 256)
        bth = (theta_b - theta_w @ (mu_c[b] * rstd_c[b])) * s0   # (256,)
        phw = (phi_w * rstd_s[b][None, :]).T.copy()
        bph = phi_b - phi_w @ (mu_s[b] * rstd_s[b])
        bout = W_w @ g_b + W_b                                   # (512,)
        in_maps.append({
            "cw": np.ascontiguousarray(
                cf[b][:, win_map].reshape(4, 128, WIN)),
            "st": np.ascontiguousarray(
                sf[b][:, ext_map].reshape(4, 128, EXT)),
            "fu": np.ascontiguousarray(ff[b].reshape(4, 128, HW)),
            "thw": np.ascontiguousarray(thw.reshape(4, 128, IC)),
            "phw": np.ascontiguousarray(phw.reshape(4, 128, IC)),
            "gw": np.ascontiguousarray(g_w.T.reshape(4, 128, IC)),
            "ww": np.ascontiguousarray(W_w.T.reshape(2, 128, C)),
            "bth": bth.astype(np.float32).reshape(2, 128, 1),
            "bph": bph.astype(np.float32).reshape(2, 128, 1),
            "bout": bout.astype(np.float32).reshape(4, 128, 1),
        })
    return in_maps


def kernel(**inputs):
    global _PROG
    if _PROG is None:
        _PROG = _build_program()
    in_maps = _host_prep(inputs)
    res = run_bass_kernel_spmd(_PROG, in_maps, core_ids=list(range(8)))
    out = np.empty((B, C, HW), np.float32)
    for r in range(8):
        b, sh = divmod(r, 4)
        out[b][:, sh * L:(sh + 1) * L] = res.results[r]["o"].reshape(C, L)
    return out.reshape(B, C, H, Wd)



# revision 2
# speedup vs baseline: 5.8667x; 5.8667x over previous
"""Trainium2 Bass kernel for nn_Nonlocal (sparse_attention, non-local style attention).

Math (per batch b):
  xn  = instance_norm(content);  sn = instance_norm(style)
  Th  = theta_w @ xn + theta_b          (256, 4096)
  Ph  = phi_w   @ sn + phi_b            (256, 4096)
  g   = g_w @ fusion_style + g_b        (256, 4096)
  f[l,m] = sum_k scale[k]^2 * <Th[:, N_k(l)], Ph[:, N_k(m)]>   (4096, 4096)
           where N_k = 3x3 reflect-padded neighborhood shift
  P = softmax_rows(f);  y = P @ g^T;  out = W_w @ y^T + W_b    (512, 4096)

The wall-clock bottleneck is the axon tunnel (~70 MB/s), so the 1x1 convs
(theta/phi/g and the final W) run on the host BLAS and only fp16 activations
are shipped:
  per core: theta window (2,128,1152), phi slice (2,128,1024), g^T slice
  (8,128,256) -- ~1.6 MB fp16. phi/g slices are AllGathered on device across
  each batch's 4-core group. Device computes f (fp16 matmuls, f32 PSUM),
  flash softmax over four 1024-col quarters, and P@g^T; returns y^T fp16.

Sharding: 8 cores = 2 batches x 4 query-row shards (1024 rows of f each).
The 3x3 shifts are folded into matmul access patterns: j-axis (within-64 with
reflection) via shifted SBUF copies, i-axis (+-64) via column offsets over
reflect-extended key windows.
"""
import numpy as np

import concourse.bass as bass
import concourse.mybir as mybir
from concourse import bacc
from concourse.bass_utils import run_bass_kernel_spmd
from concourse.tile import TileContext
from concourse.masks import make_identity

F32 = mybir.dt.float32
FP16 = mybir.dt.float16

B, C, H, Wd = 2, 512, 64, 64
HW = H * Wd          # 4096
IC = 256
L = HW // 4          # 1024 query rows per core
WIN = L + 2 * 64     # 1152 theta window cols
EXT = HW + 2 * 64    # 4224 phi extended cols
NT = L // 128        # 8 query tiles per core
NQ = 4               # psum quarters per tile (1024 key cols each)
QC = HW // NQ        # 1024

GROUPS = [[0, 1, 2, 3], [4, 5, 6, 7]]


def _jshift_copies(nc, buf, oc):
    """Fill buf[:, oc, 0/2, :] with the within-64-block reflect-shifted
    copies of buf[:, oc, 1, :]."""
    src = buf[:, oc, 1, :].rearrange("p (b j) -> p b j", j=64)
    for dj, dst_i in ((0, 0), (2, 2)):
        dst = buf[:, oc, dst_i, :].rearrange("p (b j) -> p b j", j=64)
        if dj == 0:
            nc.vector.tensor_copy(dst[:, :, 1:64], src[:, :, 0:63])
            nc.scalar.copy(dst[:, :, 0:1], src[:, :, 1:2])
        else:
            nc.vector.tensor_copy(dst[:, :, 0:63], src[:, :, 1:64])
            nc.scalar.copy(dst[:, :, 63:64], src[:, :, 62:63])


def _build_program(gather=True):
    nc = bacc.Bacc("TRN2", target_bir_lowering=False, debug=False, num_devices=8)

    th_d = nc.dram_tensor("th", [2, 128, WIN], FP16, kind="ExternalInput")
    if gather:
        ph_d = nc.dram_tensor("ph", [2, 128, L], FP16, kind="ExternalInput")
        gt_d = nc.dram_tensor("gt", [8, 128, IC], FP16, kind="ExternalInput")
    else:
        ph_d = nc.dram_tensor("ph", [2, 128, HW], FP16, kind="ExternalInput")
        gt_d = nc.dram_tensor("gt", [32, 128, IC], FP16, kind="ExternalInput")
    o_d = nc.dram_tensor("o", [NT, 128, IC], FP16, kind="ExternalOutput")

    with TileContext(nc) as tc:
        with tc.tile_pool(name="const", bufs=1) as constp, \
             tc.tile_pool(name="persist", bufs=1) as persist, \
             tc.tile_pool(name="work", bufs=2) as work, \
             tc.tile_pool(name="stats", bufs=3) as stats, \
             tc.tile_pool(name="dram", bufs=1, space="DRAM") as dram, \
             tc.tile_pool(name="fqp", bufs=2, space="PSUM") as fqp, \
             tc.tile_pool(name="ptp", bufs=2, space="PSUM") as ptp, \
             tc.tile_pool(name="yp", bufs=2, space="PSUM") as yp:

            ident = constp.tile([128, 128], F32)
            make_identity(nc, ident)

            th_j = persist.tile([128, 2, 3, WIN], FP16)   # theta, j-shifted x3
            ph_j = persist.tile([128, 2, 3, EXT], FP16)   # phi, j-shifted x3
            gt = persist.tile([128, 32, IC], FP16)        # g^T chunks (m-part)

            for oc in range(2):
                nc.sync.dma_start(out=th_j[:, oc, 1, :], in_=th_d[oc])

            if gather:
                ph_in = dram.tile([2, 128, L], FP16)
                ph_out = dram.tile([4, 2, 128, L], FP16)
                gt_in = dram.tile([8, 128, IC], FP16)
                gt_out = dram.tile([4, 8, 128, IC], FP16)
                nc.gpsimd.dma_start(out=ph_in[:], in_=ph_d[:])
                nc.gpsimd.dma_start(out=gt_in[:], in_=gt_d[:])
                nc.gpsimd.collective_compute(
                    "AllGather", mybir.AluOpType.bypass, replica_groups=GROUPS,
                    ins=[ph_in.opt()], outs=[ph_out.opt()])
                nc.gpsimd.collective_compute(
                    "AllGather", mybir.AluOpType.bypass, replica_groups=GROUPS,
                    ins=[gt_in.opt()], outs=[gt_out.opt()])
                for sh in range(4):
                    for oc in range(2):
                        nc.sync.dma_start(
                            out=ph_j[:, oc, 1, 64 + L * sh:64 + L * (sh + 1)],
                            in_=ph_out[sh, oc])
                    for ch in range(8):
                        nc.sync.dma_start(out=gt[:, 8 * sh + ch, :],
                                          in_=gt_out[sh, ch])
            else:
                for oc in range(2):
                    nc.sync.dma_start(out=ph_j[:, oc, 1, 64:64 + HW],
                                      in_=ph_d[oc])
                for ch in range(32):
                    nc.sync.dma_start(out=gt[:, ch, :], in_=gt_d[ch])

            # phi reflect extension: left ext = image cols [64,128),
            # right ext = image cols [3968,4032)
            for oc in range(2):
                nc.scalar.copy(ph_j[:, oc, 1, 0:64], ph_j[:, oc, 1, 128:192])
                nc.scalar.copy(ph_j[:, oc, 1, EXT - 64:EXT],
                               ph_j[:, oc, 1, EXT - 192:EXT - 128])
            for oc in range(2):
                _jshift_copies(nc, ph_j, oc)
                _jshift_copies(nc, th_j, oc)

            # ---- main loop over 8 query tiles ----
            for t in range(NT):
                negM = stats.tile([128, 1], F32, tag="negM")
                s_run = stats.tile([128, 1], F32, tag="s_run")
                y_sb = work.tile([128, IC], F32, tag="y_sb")
                for q in range(NQ):
                    fq = fqp.tile([128, QC], F32, tag="fq")
                    for nn in range(2):
                        cs = slice(512 * nn, 512 * (nn + 1))
                        first = True
                        for dj in range(3):
                            for di in range(3):
                                for cc in range(2):
                                    last = (dj == 2 and di == 2 and cc == 1)
                                    nc.tensor.matmul(
                                        fq[:, cs],
                                        th_j[:, cc, dj, 128 * t + 64 * di:
                                             128 * t + 64 * di + 128],
                                        ph_j[:, cc, dj, 64 * di + QC * q + 512 * nn:
                                             64 * di + QC * q + 512 * (nn + 1)],
                                        start=first, stop=last)
                                    first = False
                    # flash-style softmax over quarters
                    negmq = stats.tile([128, 1], F32, tag="negmq")
                    nc.vector.tensor_reduce(negmq, fq, axis=mybir.AxisListType.X,
                                            op=mybir.AluOpType.max, negate=True)
                    sq = stats.tile([128, 1], F32, tag="sq")
                    pq = work.tile([128, QC], F32, tag="pq")
                    if q == 0:
                        nc.vector.tensor_copy(negM, negmq)
                        nc.scalar.activation(pq, fq, mybir.ActivationFunctionType.Exp,
                                             bias=negM, scale=1.0, accum_out=s_run)
                    else:
                        posM_old = stats.tile([128, 1], F32, tag="posM")
                        nc.vector.tensor_scalar_mul(posM_old, negM, -1.0)
                        nc.vector.tensor_tensor(negM, negM, negmq,
                                                op=mybir.AluOpType.min)
                        cfac = stats.tile([128, 1], F32, tag="cfac")
                        nc.scalar.activation(cfac, negM,
                                             mybir.ActivationFunctionType.Exp,
                                             bias=posM_old, scale=1.0)
                        nc.scalar.activation(pq, fq, mybir.ActivationFunctionType.Exp,
                                             bias=negM, scale=1.0, accum_out=sq)
                        nc.vector.tensor_scalar_mul(s_run, s_run, cfac)
                        nc.vector.tensor_tensor(s_run, s_run, sq,
                                                op=mybir.AluOpType.add)
                        nc.vector.tensor_scalar_mul(y_sb, y_sb, cfac)
                    # transpose P quarter + PV partial
                    y_ps = yp.tile([128, IC], F32, tag="yps")
                    ptsb = work.tile([128, 8, 128], FP16, tag="ptsb")
                    for j in range(8):
                        pt_ps = ptp.tile([128, 128], F32, tag="pt")
                        nc.tensor.transpose(pt_ps, pq[:, 128 * j:128 * (j + 1)], ident)
                        if j % 2 == 0:
                            nc.vector.tensor_copy(ptsb[:, j, :], pt_ps)
                        else:
                            nc.scalar.copy(ptsb[:, j, :], pt_ps)
                    for j in range(8):
                        nc.tensor.matmul(y_ps, ptsb[:, j, :], gt[:, 8 * q + j, :],
                                         start=(j == 0), stop=(j == 7))
                    if q == 0:
                        nc.vector.tensor_copy(y_sb, y_ps)
                    else:
                        nc.vector.tensor_tensor(y_sb, y_sb, y_ps,
                                                op=mybir.AluOpType.add)
                # normalize and emit y^T tile in fp16
                rec = stats.tile([128, 1], F32, tag="rec")
                nc.vector.reciprocal(rec, s_run)
                yn = work.tile([128, IC], FP16, tag="yn")
                nc.vector.tensor_scalar_mul(yn, y_sb, rec)
                nc.sync.dma_start(out=o_d[t], in_=yn)

    nc.compile()
    return nc


_PROG = None
_USE_CC = True


def _host_prep(inputs):
    EPS = 1e-5
    content = np.asarray(inputs["content"], np.float32).reshape(B, C, HW)
    style = np.asarray(inputs["style"], np.float32).reshape(B, C, HW)
    fusion = np.asarray(inputs["fusion_style"], np.float32).reshape(B, C, HW)
    theta_w = np.asarray(inputs["theta_w"], np.float32)
    theta_b = np.asarray(inputs["theta_b"], np.float32)
    phi_w = np.asarray(inputs["phi_w"], np.float32)
    phi_b = np.asarray(inputs["phi_b"], np.float32)
    g_w = np.asarray(inputs["g_w"], np.float32)
    g_b = np.asarray(inputs["g_b"], np.float32)
    scale = np.asarray(inputs["scale"], np.float32)

    s2 = scale.astype(np.float64) ** 2
    if not np.allclose(s2, s2[0]):
        raise NotImplementedError("non-uniform ContextAtten scale not supported")
    s0 = float(s2[0])

    in_maps = []
    for b in range(B):
        cf, sf, ff = content[b], style[b], fusion[b]
        mu_c = cf.mean(-1)
        rc = 1.0 / np.sqrt(cf.var(-1, ddof=1) + EPS)
        mu_s = sf.mean(-1)
        rs = 1.0 / np.sqrt(sf.var(-1, ddof=1) + EPS)

        # fold instance norm (and uniform scale**2 on theta) into the convs
        thA = theta_w * (rc * s0)[None, :]
        bth = (theta_b - theta_w @ (mu_c * rc)) * s0
        phA = phi_w * rs[None, :]
        bph = phi_b - phi_w @ (mu_s * rs)

        Th = thA @ cf
        Th += bth[:, None]
        Ph = phA @ sf
        Ph += bph[:, None]
        G = g_w @ ff
        G += g_b[:, None]

        # reflect extension on theta (i-axis): ext cols = [64:128] | all | [3968:4032]
        Th_e = np.concatenate(
            [Th[:, 64:128], Th, Th[:, HW - 128:HW - 64]], axis=1).astype(np.float16)
        Ph16 = Ph.astype(np.float16)
        GT16 = G.T.astype(np.float16)  # (4096, 256)

        for sh in range(4):
            q0 = sh * L
            m = {
                "th": np.ascontiguousarray(
                    Th_e[:, q0:q0 + WIN]).reshape(2, 128, WIN),
                "ph": np.ascontiguousarray(
                    Ph16[:, q0:q0 + L]).reshape(2, 128, L),
                "gt": GT16[q0:q0 + L].reshape(8, 128, IC),
            }
            if not _USE_CC:
                m["ph"] = Ph16.reshape(2, 128, HW)
                m["gt"] = GT16.reshape(32, 128, IC)
            in_maps.append(m)
    return in_maps


def kernel(**inputs):
    global _PROG
    if _PROG is None:
        _PROG = _build_program(gather=_USE_CC)
    in_maps = _host_prep(inputs)
    res = run_bass_kernel_spmd(_PROG, in_maps, core_ids=list(range(8)))

    W_w = np.asarray(inputs["W_w"], np.float32)
    W_b = np.asarray(inputs["W_b"], np.float32)
    out = np.empty((B, C, HW), np.float32)
    for b in range(B):
        yT = np.concatenate(
            [res.results[4 * b + sh]["o"].reshape(L, IC) for sh in range(4)],
            axis=0).astype(np.float32)  # (4096, 256)
        out[b] = W_w @ yT.T
        out[b] += W_b[:, None]
    return out.reshape(B, C, H, Wd)
